# revision 20
# baseline (speedup 1.0000x reference)
"""GroupQueryAttention (16 heads, hd=128) on 8 trn2 cores, heads sharded 2/core.

All tensor data fp16 (f32 PSUM accumulation). Per core c (2 heads):
  xT   [B, 4, 128, 16, 512] fp16  x[b].T: (chunk, partition, h-ktile, t-col);
                                  per-partition contiguous 16KB -> 128-desc DMA
  wqT/wkT/wvT [128, 16, 256] fp16 W[256c:256c+256,:].T, partition-major
  woT  [128, 2, 2048] fp16        Wo[:, 256c:256c+256].T per local head
  out  [4096, 2048] fp16          partial product, host sums over cores

Schedule: batch-0 projections; then batch-0 attention chunks with batch-1
projection chunks interleaved into the PE queue (attention inner loop is
ACT-bound at ~612ns/ktile vs PE ~426ns, so independent projection matmuls
absorb the PE idle); out-projections lag one chunk so they fill the next
chunk's ACT-bound window. Rowsum via DVE pair/quad adds + ones-col matmuls;
normalization fused into the PSUM->SBUF copy (scalar_tensor_tensor).
"""
import sys

for _p in ("/opt/trn_rl_repo",):
    if _p not in sys.path:
        sys.path.insert(0, _p)

import numpy as np

import concourse.bass as bass
import concourse.tile as tile
from concourse import bacc, mybir
from concourse.bass_utils import run_bass_kernel_spmd

N_CORES = 8
B, T, H = 2, 2048, 2048
NH, HD = 16, 128
HPC = H // N_CORES          # 256 dims (2 heads) per core
HEADS_PC = NH // N_CORES    # 2
KT = H // 128               # 16 k-tiles along hidden
TCH = 4                     # t-chunks (512 cols) per batch for projections
TC = T // TCH               # 512
QC = 512                    # tq chunk in attention
NQC = T // QC               # 4
SCALE = float(HD) ** -0.5

F32 = mybir.dt.float32
F32R = mybir.dt.float32r
F16 = mybir.dt.float16
AF = mybir.ActivationFunctionType
OP = mybir.AluOpType


def r(ap):
    return ap


_CACHE = {}


def _build(use_mask, use_bias, reps=1):
    key = (use_mask, use_bias, reps)
    if key in _CACHE:
        return _CACHE[key]

    nc = bacc.Bacc("TRN2", target_bir_lowering=False, debug=False,
                   num_devices=N_CORES)
    xT = nc.dram_tensor("xT", [B, TCH, 128, KT, TC], F16, kind="ExternalInput").ap()
    wqT = nc.dram_tensor("wqT", [128, KT, HPC], F16, kind="ExternalInput").ap()
    wkT = nc.dram_tensor("wkT", [128, KT, HPC], F16, kind="ExternalInput").ap()
    wvT = nc.dram_tensor("wvT", [128, KT, HPC], F16, kind="ExternalInput").ap()
    woT = nc.dram_tensor("woT", [128, HEADS_PC, H], F16, kind="ExternalInput").ap()
    if use_bias:
        bqd = nc.dram_tensor("bq", [HEADS_PC, 128], F32, kind="ExternalInput").ap()
        bkd = nc.dram_tensor("bk", [HEADS_PC, 128], F32, kind="ExternalInput").ap()
        bvd = nc.dram_tensor("bv", [1, HPC], F32R, kind="ExternalInput").ap()
    if use_mask:
        # mask[b,0].T / SCALE, tk-tiled
        mkd = nc.dram_tensor("maskT", [B, KT, 128, T], F32, kind="ExternalInput").ap()
    onr = nc.dram_tensor("ones_row", [1, 128], F32R, kind="ExternalInput").ap()
    out = nc.dram_tensor("out", [B * T, H], F16, kind="ExternalOutput").ap()

    from contextlib import ExitStack
    with tile.TileContext(nc) as tc, ExitStack() as ctx:
        wpool = ctx.enter_context(tc.tile_pool(name="wts", bufs=1))
        cpool = ctx.enter_context(tc.tile_pool(name="consts", bufs=1))
        xpool = ctx.enter_context(tc.tile_pool(name="xt", bufs=2))
        qkv_pool = ctx.enter_context(tc.tile_pool(name="qkv", bufs=1))
        pr_pool = ctx.enter_context(tc.tile_pool(name="probs", bufs=4))
        acc_pool = ctx.enter_context(tc.tile_pool(name="acc", bufs=2))
        rec_pool = ctx.enter_context(tc.tile_pool(name="rec", bufs=2))
        bcs_pool = ctx.enter_context(tc.tile_pool(name="bcs", bufs=2))
        at_pool = ctx.enter_context(tc.tile_pool(name="attnT", bufs=1))
        os_pool = ctx.enter_context(tc.tile_pool(name="osb", bufs=3))
        if use_mask:
            mk_pool = ctx.enter_context(tc.tile_pool(name="mask", bufs=4))

        proj_ps = ctx.enter_context(tc.tile_pool(name="proj_ps", bufs=2, space="PSUM"))
        sc_ps = ctx.enter_context(tc.tile_pool(name="sc_ps", bufs=2, space="PSUM"))
        pv_ps = ctx.enter_context(tc.tile_pool(name="pv_ps", bufs=2, space="PSUM"))
        rb_ps = ctx.enter_context(tc.tile_pool(name="rb_ps", bufs=1, space="PSUM"))

        for _rep in range(reps):
            # ---- load weights / constants (first blocks split for fast start) ----
            # first chunk + wq arrive in 4-ktile pieces so proj matmuls start
            # as soon as piece 0 lands
            xt00 = xpool.tile([128, KT * TC], F16, tag="xt", name="xt00")
            wq = wpool.tile([128, KT * HPC], F16, tag="wqT", name="wq_t")
            for g in range(0, KT, 4):
                nc.sync.dma_start(
                    wq[:, g * HPC:(g + 4) * HPC].rearrange(
                        "p (i j) -> p i j", j=HPC), wqT[:, g:g + 4])
                nc.sync.dma_start(
                    xt00[:, g * TC:(g + 4) * TC].rearrange(
                        "p (i j) -> p i j", j=TC), xT[0, 0, :, g:g + 4])
            wk = wpool.tile([128, KT * HPC], F16, tag="wkT")
            nc.sync.dma_start(wk[:].rearrange("p (i j) -> p i j", j=HPC), wkT)
            wv = wpool.tile([128, KT * HPC], F16, tag="wvT")
            nc.sync.dma_start(wv[:].rearrange("p (i j) -> p i j", j=HPC), wvT)
            wo = wpool.tile([128, HEADS_PC * H], F16, tag="wo")

            ones_col = cpool.tile([128, 1], F16, tag="ones_col")
            nc.vector.memset(ones_col[:], 1.0)
            ones_row = cpool.tile([1, 128], F32R, tag="ones_row")
            nc.sync.dma_start(ones_row[:], onr)

            if use_bias:
                bq_t = cpool.tile([128, HEADS_PC], F32, tag="bq")
                nc.sync.dma_start(bq_t[:], bqd.rearrange("h p -> p h"))
                bk_t = cpool.tile([128, HEADS_PC], F32, tag="bk")
                nc.sync.dma_start(bk_t[:], bkd.rearrange("h p -> p h"))
                bv_row = cpool.tile([1, HPC], F32R, tag="bv_row")
                nc.sync.dma_start(bv_row[:], bvd)
                bv_ps = rb_ps.tile([128, HPC], F32, tag="rb")
                nc.tensor.matmul(bv_ps[:], r(ones_row[:]), r(bv_row[:]),
                                 start=True, stop=True)
                bv_bc = cpool.tile([128, HPC], F32, tag="bv_bc")
                nc.vector.tensor_copy(bv_bc[:], bv_ps[:])

            def alloc_qkv():
                return {
                    "q": [qkv_pool.tile([128, T], F16, tag=f"q{h}", bufs=2,
                                        name=f"qT{h}")
                          for h in range(HEADS_PC)],
                    "k": [qkv_pool.tile([128, T], F16, tag=f"k{h}", bufs=2,
                                        name=f"kT{h}")
                          for h in range(HEADS_PC)],
                    "v": qkv_pool.tile([128, KT * HPC], F16, tag="v", bufs=2,
                                       name="vt"),
                }

            def proj_chunk(b, c, qkv, first_xt=None):
                """q/k/v projection for 512 tokens (chunk c) of batch b."""
                if first_xt is not None:
                    xt = first_xt
                else:
                    xt = xpool.tile([128, KT * TC], F16, tag="xt")
                    nc.sync.dma_start(xt[:].rearrange("p (i j) -> p i j", j=TC),
                                      xT[b, c])
                ncopy = 0
                for h in range(HEADS_PC):
                    for w_, dst, bias_t in ((wq, qkv["q"][h], "bq"),
                                            (wk, qkv["k"][h], "bk")):
                        ps = proj_ps.tile([128, TC], F32, tag="proj")
                        for i in range(KT):
                            nc.tensor.matmul(
                                ps[:],
                                r(w_[:, i * HPC + 128 * h: i * HPC + 128 * h + 128]),
                                r(xt[:, i * TC: (i + 1) * TC]),
                                start=(i == 0), stop=(i == KT - 1))
                        sl = dst[:, c * TC:(c + 1) * TC]
                        if use_bias:
                            bt = bq_t if bias_t == "bq" else bk_t
                            nc.scalar.activation(sl, ps[:], AF.Identity,
                                                 bias=bt[:, h:h + 1])
                        else:
                            if ncopy % 2 == 0:
                                nc.vector.tensor_copy(sl, ps[:])
                            else:
                                nc.scalar.copy(sl, ps[:])
                            ncopy += 1
                for s in range(TC // 128):  # 128-row t-subtiles of this chunk
                    tt = (TC // 128) * c + s
                    ps = proj_ps.tile([128, HPC], F32, tag="proj")
                    for i in range(KT):
                        nc.tensor.matmul(
                            ps[:],
                            r(xt[:, i * TC + 128 * s: i * TC + 128 * s + 128]),
                            r(wv[:, i * HPC: (i + 1) * HPC]),
                            start=(i == 0), stop=(i == KT - 1))
                    vsl = qkv["v"][:, tt * HPC:(tt + 1) * HPC]
                    if use_bias:
                        nc.vector.scalar_tensor_tensor(
                            vsl, ps[:], 1.0, bv_bc[:], op0=OP.mult, op1=OP.add)
                    else:
                        if ncopy % 2 == 0:
                            nc.vector.tensor_copy(vsl, ps[:])
                        else:
                            nc.scalar.copy(vsl, ps[:])
                        ncopy += 1

            def attn_chunk(b, ch, qkv, pool_pairs=False):
                """softmax(q.T k / sqrt(hd)) v for 512 queries; returns attnT."""
                attnT = [at_pool.tile([128, QC], F16, tag=f"a{h}", bufs=2,
                                      name=f"attnT{h}")
                         for h in range(HEADS_PC)]
                for h in range(HEADS_PC):
                    q_sl = r(qkv["q"][h][:, ch * QC:(ch + 1) * QC])
                    kTt, vt = qkv["k"][h], qkv["v"]
                    pv = pv_ps.tile([128, QC], F32, tag="pv")
                    rs = rb_ps.tile([1, QC], F32, tag="rb")
                    prs, scs = [], []
                    pair = None
                    for i in range(KT + 1):
                        if i < KT:  # score matmul one k-tile ahead of exp/pv
                            sc = sc_ps.tile([128, QC], F32, tag="sc")
                            nc.tensor.matmul(sc[:],
                                             r(kTt[:, i * 128:(i + 1) * 128]),
                                             q_sl, start=True, stop=True)
                            if use_mask:
                                mk = mk_pool.tile([128, QC], F32, tag="mk")
                                nc.sync.dma_start(
                                    mk[:], mkd[b, i, :, ch * QC:(ch + 1) * QC])
                                nc.vector.tensor_add(sc[:], sc[:], mk[:])
                            scs.append(sc)
                        if i == 0:
                            continue
                        j = i - 1
                        pr = pr_pool.tile([128, QC], F16, tag="pr")
                        nc.scalar.activation(pr[:], scs[j][:], AF.Exp, scale=SCALE)
                        nc.tensor.matmul(
                            pv[:],
                            r(vt[:, j * HPC + 128 * h: j * HPC + 128 * h + 128]),
                            r(pr[:]), start=(j == 0), stop=(j == KT - 1))
                        prs.append(pr)
                        eng_add = nc.gpsimd if pool_pairs else nc.vector
                        if j % 4 == 1:
                            pair = acc_pool.tile([128, QC], F16, tag="acc")
                            eng_add.tensor_add(pair[:], prs[j - 1][:], prs[j][:])
                        elif j % 4 == 3:
                            pair2 = acc_pool.tile([128, QC], F16, tag="acc2")
                            eng_add.tensor_add(pair2[:], prs[j - 1][:], prs[j][:])
                            quad = acc_pool.tile([128, QC], F16, tag="acc")
                            nc.vector.tensor_add(quad[:], pair[:], pair2[:])
                            nc.tensor.matmul(rs[:], r(ones_col[:]), r(quad[:]),
                                             start=(j == 3), stop=(j == KT - 1))
                    rec = rec_pool.tile([1, QC], F32R, tag="rec")
                    with nc.allow_low_precision(reason="f32r rowsum reciprocal"):
                        nc.vector.reciprocal(rec[:], rs[:])
                    bc = rb_ps.tile([128, QC], F32, tag="rb")
                    nc.tensor.matmul(bc[:], r(ones_row[:]), r(rec[:]),
                                     start=True, stop=True)
                    # hw verifier: DVE may read only one non-scalar PSUM input,
                    # so bc must be staged through SBUF before the normalize
                    bcs = bcs_pool.tile([128, QC], F32, tag="bcs")
                    nc.vector.tensor_copy(bcs[:], bc[:])
                    nc.vector.scalar_tensor_tensor(
                        attnT[h][:], pv[:], 1.0, bcs[:], op0=OP.mult, op1=OP.mult)
                return attnT

            def outproj_chunk(b, ch, attnT, engines, spare_bank=False):
                """partial out (local heads) for chunk ch's 4 t-tiles.

                spare_bank: alternate PSUM with the (idle) proj tag so the
                PSUM->SBUF copies double-buffer; only valid once projections
                are done (batch-1 windows)."""
                for st in range(4):
                    tt = 4 * ch + st
                    ob = os_pool.tile([128, H], F16, tag="ob")
                    for oc in range(4):
                        if spare_bank and oc % 2 == 1:
                            ps = proj_ps.tile([128, 512], F32, tag="proj")
                        else:
                            ps = proj_ps.tile([128, 512], F32, tag="oproj", bufs=1)
                        for h in range(HEADS_PC):
                            nc.tensor.matmul(
                                ps[:],
                                r(attnT[h][:, st * 128:(st + 1) * 128]),
                                r(wo[:, h * H + oc * 512: h * H + (oc + 1) * 512]),
                                start=(h == 0), stop=(h == HEADS_PC - 1))
                        eng = engines[(st * 4 + oc) % len(engines)]
                        osl = ob[:, oc * 512:(oc + 1) * 512]
                        if eng == "v":
                            nc.vector.tensor_copy(osl, ps[:])
                        elif eng == "s":
                            nc.scalar.copy(osl, ps[:])
                        else:
                            nc.gpsimd.tensor_copy(osl, ps[:])
                    nc.gpsimd.dma_start(
                        out[b * T + tt * 128: b * T + (tt + 1) * 128, :], ob[:])

            # ---- schedule ----
            qkv0 = alloc_qkv()
            for c in range(TCH):
                proj_chunk(0, c, qkv0, first_xt=xt00 if c == 0 else None)
            nc.sync.dma_start(wo[:].rearrange("p (i j) -> p i j", j=H), woT)

            qkv1 = alloc_qkv()
            at_prev = None
            for ch in range(NQC):      # batch-0 attention window
                at = attn_chunk(0, ch, qkv0)
                proj_chunk(1, ch, qkv1)
                if at_prev is not None:
                    outproj_chunk(0, ch - 1, at_prev, engines=("v", "s"))
                at_prev = at
            for ch in range(NQC):      # batch-1 attention window
                at = attn_chunk(1, ch, qkv1)
                outproj_chunk(1 if ch else 0, ch - 1 if ch else NQC - 1, at_prev,
                              engines=("v",), spare_bank=True)
                at_prev = at
            outproj_chunk(1, NQC - 1, at_prev, engines=("v", "s"),
                          spare_bank=True)

    nc.compile()
    _CACHE[key] = nc
    return nc


def prepare(inputs):
    hs = np.asarray(inputs["hidden_states"], dtype=np.float16)
    mask = np.asarray(inputs["attention_mask"], dtype=np.float32)
    Wq = np.asarray(inputs["Wq"], dtype=np.float32)
    Wk = np.asarray(inputs["Wk"], dtype=np.float32)
    Wv = np.asarray(inputs["Wv"], dtype=np.float32)
    Wo = np.asarray(inputs["Wo"], dtype=np.float32)
    bq = np.asarray(inputs["bq"], dtype=np.float32)
    bk = np.asarray(inputs["bk"], dtype=np.float32)
    bv = np.asarray(inputs["bv"], dtype=np.float32)

    use_mask = bool(np.any(mask))
    use_bias = bool(np.any(bq) or np.any(bk) or np.any(bv))
    nc = _build(use_mask, use_bias)

    # x[b].T -> [KT,128,TCH,TC] -> [TCH,128,KT,TC]; per-partition contiguous
    xTh = np.stack([
        np.ascontiguousarray(
            hs[b].T.reshape(KT, 128, TCH, TC).transpose(2, 1, 0, 3))
        for b in range(B)])

    def wslice(W, sl):
        return np.ascontiguousarray(
            W[sl].T.astype(np.float16).reshape(KT, 128, HPC).transpose(1, 0, 2))

    in_maps = []
    for c in range(N_CORES):
        sl = slice(c * HPC, (c + 1) * HPC)
        m = {
            "ones_row": np.ones((1, 128), np.float32),
            "xT": xTh,
            "wqT": wslice(Wq, sl),
            "wkT": wslice(Wk, sl),
            "wvT": wslice(Wv, sl),
            "woT": np.ascontiguousarray(
                Wo[:, sl].T.astype(np.float16).reshape(HEADS_PC, 128, H)
                .transpose(1, 0, 2)),
        }
        if use_bias:
            m["bq"] = np.ascontiguousarray(bq[sl]).reshape(HEADS_PC, 128)
            m["bk"] = np.ascontiguousarray(bk[sl]).reshape(HEADS_PC, 128)
            m["bv"] = np.ascontiguousarray(bv[sl]).reshape(1, HPC)
        if use_mask:
            mt = mask[:, 0].transpose(0, 2, 1) / SCALE  # [B, tk, tq]
            m["maskT"] = np.ascontiguousarray(mt).reshape(B, KT, 128, T)
        in_maps.append(m)
    return nc, in_maps


def build_for_timing(inputs, reps):
    """Same kernel flags as prepare(), body repeated `reps` times in one NEFF."""
    mask = np.asarray(inputs["attention_mask"], dtype=np.float32)
    bq = np.asarray(inputs["bq"], dtype=np.float32)
    bk = np.asarray(inputs["bk"], dtype=np.float32)
    bv = np.asarray(inputs["bv"], dtype=np.float32)
    use_mask = bool(np.any(mask))
    use_bias = bool(np.any(bq) or np.any(bk) or np.any(bv))
    return _build(use_mask, use_bias, reps=reps)


def postprocess(results, inputs):
    bo = np.asarray(inputs["bo"], dtype=np.float32)
    acc = results[0]["out"].astype(np.float32)
    for c in range(1, N_CORES):
        acc = acc + results[c]["out"].astype(np.float32)
    return (acc + bo).reshape(B, T, H)


def kernel(**inputs):
    nc, in_maps = prepare(inputs)
    res = run_bass_kernel_spmd(nc, in_maps, list(range(N_CORES)))
    return postprocess(res.results, inputs)
